# revision 8
# baseline (speedup 1.0000x reference)
"""Trainium2 Bass kernel for ExemplarGNN2AdjModel (gnn_message_passing).

Math:
  h  = relu(relu(x@W1+b1)@W2+b2)                      # [512,128] node encoder
  scores[i,j] = Wp2 . relu(Wp1a.h_i + Wp1b.h_j + Wp1c.|h_i-h_j| + bp1) + bp2

Device algorithm (per core, SPMD over 8 cores; core c handles 64 rows of i):
  - Each core receives x pre-rolled by c*64 rows and pre-transposed (xT), so the
    identical program computes rows [c*64, c*64+64) in its local (rolled) node
    order; the host un-rolls the output columns afterwards.
  - |h_i - h_j| = h_i + h_j - 2*min(h_i,h_j): the h_i term is folded into the
    per-i bias (wp1a += w3), the h_j term into the B matmul (w2p += w3), and
    the per-pair part is -2*w3^T min(h_i, h_j) =: w3^T d_r.
  - Two weight-space folds remove per-row work:
      M = (w2p w3^{-1})^T h   satisfies  w3^T M = w2p^T h        (the E term)
      C = (wp1a w3^{-1})^T h + w3^{-T} bp1  satisfies  w3^T C_r = A2_r (bias)
    so   P_r + A2_r = w3^T (min(h, h_r) + C_r [+ M]).
    C_r is a per-partition scalar -> rides in tensor_scalar's SECOND alu slot
    for free; every relu becomes BIASLESS and batchable.  M is a full tensor;
    rows that carry it (via a tensor_tensor add) skip their w2p matmul.
    Values reach ~25 in magnitude, so everything is fp16 (10 mantissa bits;
    same PE/DVE rate as bf16).  Measured end-to-end rel err ~1.8e-3.
  - Per 4-row group (rows g, g+16, g+32, g+48):
      r0,r1 (duoA): classic two-matmul accumulate (w2p start + w3 stop),
                    d = (h min h_r) add C_r   -- one DVE tensor_scalar each.
      r2,r3 (duoB): fold rows: dp = ((h min h_r) add C_r) then add M
                    (DVE tensor_scalar + tensor_tensor), single w3 matmul.
      PE order: w2pA0, w2pA1, w3B0, [deferred outs g-2], w3B1, w3A0, w3A1
      relus: each duo is ONE biasless ACT op over its [128,1024] two-bank
             PSUM tile (duoB first - its matmuls finish earlier).
    GPSIMD runs nothing in steady state: its elementwise ops (~1.2us per
    [128,512]) contend for SBUF ports and were measured inflating concurrent
    DVE ops by ~60%.
  - Encoder (fp16, fp32 PSUM), M and C matmuls run once in the ramp.
  - out[16b+i,:] += embW_r^T hid_r: 4 col-tiled PE matmuls to PSUM partitions
    32b..32b+8 of the phase bank -- disjoint col_grp strips run concurrently
    (~1 slot for all 4).  embW_r = embbuf[:, 15-i : 31-i]: a sliding window
    over a 31-column zero buffer with Wp2 at column 15 puts Wp2 exactly in
    stationary column i.  Out matmuls of group g are issued in group g+2.
    Two 8-group phases accumulate into separate banks outp0/outp1 (PH=16 in
    one bank measured +219ns/group of PE time on the out matmuls).
  - Output: bp2 is added on the host.  Phase-0 flush: PSUM->SBUF copy in two
    column-halves on DVE during groups 12-13 (DVE has slack there), DMAs on
    sync/gpsimd mid-steady.  Phase-1 flush at the tail: copy halves on
    DVE+ACT in parallel, DMAs on sync/scalar (gpsimd is kept strictly off
    the tail: its SWDGE drain costs ~2us at kernel end).
  - Startup: xtp is DMA'd in 5 k-chunks with doorbells spread across the sync/
    gpsimd queues (doorbells cost ~600ns each and serialize per queue);
    encoder matmuls start as chunks land, with narrow [0:128] first-pieces of
    relu1/h2/hbf so the hT/min chain launches early; small dummy matmuls
    bridge every DMA/relu gap so the PE HAM clock-gate stays at 2.4 GHz.
"""

import numpy as np

B = 512
IN_DIM = 595
HID = 128
NCORES = 8
RPC = B // NCORES  # rows per core = 64
NBLK = 4           # output col-tile blocks
BLK = RPC // NBLK  # 16 rows per block
DEFER_G = 2        # groups between producing hid and its out matmul
N_WARM_MM = 8      # dummy matmuls: sustained PE activity trips the HAM
                   # clock-gate to 2.4 GHz before the encoder matmuls run
WARM_N = 256       # free dim of warm matmuls

# in_dim k-tiles for the first encoder matmul (contraction over 595)
KT = [(0, 128), (128, 256), (256, 384), (384, 512), (512, 595)]

_PROGRAM_CACHE = {}


def _build_program():
    import concourse.mybir as mybir
    import concourse.tile as tile
    from concourse import bacc

    f32 = mybir.dt.float32
    f16 = mybir.dt.float16
    Act = mybir.ActivationFunctionType
    Alu = mybir.AluOpType

    nc = bacc.Bacc("TRN2", target_bir_lowering=False)

    NKT = len(KT)
    xt_d = nc.dram_tensor("xtp", [HID, NKT * B], f16, kind="ExternalInput")
    w1_d = nc.dram_tensor("w1p", [HID, NKT * HID], f16, kind="ExternalInput")
    wpack_d = nc.dram_tensor("wpack", [HID, 5 * HID], f16, kind="ExternalInput")
    bias_d = nc.dram_tensor("biases", [HID, 4], f32, kind="ExternalInput")
    out_d = nc.dram_tensor("out", [RPC, B], f32, kind="ExternalOutput")

    with tile.TileContext(nc) as tc:
        with (
            tc.tile_pool(name="consts", bufs=1) as consts,
            tc.tile_pool(name="setup", bufs=1) as setup,
            tc.tile_pool(name="dwork", bufs=12) as dwork,
            tc.tile_pool(name="hwork", bufs=8) as hwork,
            tc.tile_pool(name="penc", bufs=2, space="PSUM") as penc,
            tc.tile_pool(name="pduo", bufs=3, space="PSUM") as pduo,
        ):
            # ---- input loads first: doorbells cost ~600ns each and serialize
            # per queue, so spread the xtp chunks across three idle queues.
            xt_all = consts.tile([HID, NKT * B], f16)
            w1_all = consts.tile([HID, NKT * HID], f16)
            biases = consts.tile([HID, 4], f32)
            wpack = consts.tile([HID, 5 * HID], f16)
            # earliest-needed first; k-chunks split across sync/gpsimd queues
            nc.scalar.dma_start(out=w1_all, in_=w1_d[:, :])
            qeng = [nc.sync, nc.gpsimd, nc.sync, nc.gpsimd, nc.sync]
            for k in range(NKT):
                qeng[k].dma_start(
                    out=xt_all[:, k * B : (k + 1) * B],
                    in_=xt_d[:, k * B : (k + 1) * B],
                )
            nc.scalar.dma_start(out=biases, in_=bias_d[:, :])
            nc.scalar.dma_start(out=wpack, in_=wpack_d[:, :])

            # ---- PE warm-up over the DMA window (HAM ramps to 2.4 GHz).
            # Warm matmuls allocate full duo tiles (2 banks) but only write
            # [:, :WARM_N]; they keep the pduo ring busy only pre-loop.
            scratch = setup.tile([HID, B], f16)
            nc.vector.memset(scratch, 0.0)
            scratch1 = setup.tile([HID, 1], f32)
            nc.scalar.activation(scratch1, scratch[:, 0:1], Act.Relu)

            def warm_mm(n, w=WARM_N):
                # dummy matmuls keep the PE busy (HAM clock-gate stays at
                # 2.4 GHz) across DMA-wait and relu-wait gaps
                for _ in range(n):
                    wp = pduo.tile([HID, 2 * B], f32, name="duo")
                    nc.tensor.matmul(
                        wp[:, 0:w], lhsT=scratch[:, 0:HID], rhs=scratch[:, 0:w],
                        start=True, stop=True, skip_group_check=True,
                    )

            warm_mm(N_WARM_MM)

            # sliding-window Wp2 buffer: zeros with Wp2 at column BLK-1; the
            # out matmul for block-row i uses embbuf[:, BLK-1-i+c] == Wp2 iff
            # c == i.
            embbuf = consts.tile([HID, 2 * BLK - 1], f16)
            nc.vector.memset(embbuf, 0.0)

            xt_sb = [xt_all[:, k * B : (k + 1) * B] for k in range(NKT)]
            w1_sb = [w1_all[:, k * HID : (k + 1) * HID] for k in range(NKT)]
            w2_sb = wpack[:, 0 * HID : 1 * HID]
            w2p_sb = wpack[:, 1 * HID : 2 * HID]
            w3_sb = wpack[:, 2 * HID : 3 * HID]
            g1_sb = wpack[:, 3 * HID : 4 * HID]
            g2_sb = wpack[:, 4 * HID : 5 * HID]
            b1_sb = biases[:, 0:1]
            b2_sb = biases[:, 1:2]
            cb_sb = biases[:, 2:3]

            # ---- encoder: h1 = relu(W1^T xT + b1), hT = relu(W2^T h1 + b2) ----
            h1p = penc.tile([HID, B], f32, name="encp", tag="encp")
            for k in range(len(KT)):
                nc.tensor.matmul(
                    h1p, lhsT=w1_sb[k], rhs=xt_sb[k],
                    start=(k == 0), stop=(k == len(KT) - 1),
                )
                if k > 0:
                    warm_mm(1)  # bridge the DMA-gated gaps between k-chunks
            # encoder relus split: a narrow ACT first-piece [0:128] lets the
            # h2 -> hbf -> hT -> min chain start early; DVE takes the rest
            HQ = B // 4
            HB2 = B // 2
            h1bf = setup.tile([HID, B], f16)
            nc.scalar.activation(h1bf[:, 0:HQ], h1p[:, 0:HQ], Act.Relu, bias=b1_sb)
            nc.vector.tensor_scalar(
                h1bf[:, HQ:B], h1p[:, HQ:B], b1_sb, 0.0, Alu.add, Alu.max
            )

            # h2 in two matmuls to different PSUM banks: a narrow first piece
            # [0:128] feeding ACT, the rest feeding DVE, so the hT/min chain
            # starts as early as possible
            h2p = penc.tile([HID, HQ], f32, name="encp2", tag="encp")
            nc.tensor.matmul(h2p, lhsT=w2_sb, rhs=h1bf[:, 0:HQ], start=True, stop=True)
            h2pb = pduo.tile([HID, 2 * B], f32, name="duo")
            nc.tensor.matmul(
                h2pb[:, 0 : B - HQ], lhsT=w2_sb, rhs=h1bf[:, HQ:B],
                start=True, stop=True, skip_group_check=True,
            )
            warm_mm(2)  # bridge PE over relu2 + hT
            hbf = setup.tile([HID, B], f16)
            nc.scalar.activation(hbf[:, 0:HQ], h2p, Act.Relu, bias=b2_sb)
            nc.vector.tensor_scalar(
                hbf[:, HQ:B], h2pb[:, 0 : B - HQ], b2_sb, 0.0, Alu.add, Alu.max
            )
            # hT fp32 is the per-row scalar operand of the min (tensor_scalar
            # scalars must be fp32); only the core's 64 local-row columns are
            # ever read, and deriving it from hbf avoids a second serialized
            # read of the h2p PSUM bank
            hT = setup.tile([HID, RPC], f32)
            nc.vector.tensor_copy(hT, hbf[:, 0:RPC])
            # Wp2 rides in as f32 column 3 of biases; cast into the sliding
            # window buffer.  Emitted HERE (needed only by the first out
            # matmuls): emitting it earlier head-of-line-blocks the in-order
            # DVE queue on the biases DMA and delays the encoder relus ~1us.
            nc.vector.tensor_copy(embbuf[:, BLK - 1 : BLK], biases[:, 3:4])

            # ---- M = (w2p w3^{-1})^T h  and  C = (wp1a w3^{-1})^T h + cb ----
            mp = penc.tile([HID, B], f32, name="encp3", tag="encp")
            nc.tensor.matmul(mp, lhsT=g1_sb, rhs=hbf, start=True, stop=True)
            warm_mm(1)
            cp = penc.tile([HID, B], f32, name="encp4", tag="encp")
            nc.tensor.matmul(cp, lhsT=g2_sb, rhs=hbf, start=True, stop=True)
            warm_mm(1)  # bridge PE over the first mins
            msb = setup.tile([HID, B], f16)
            nc.vector.tensor_copy(msb, mp)
            csb = setup.tile([HID, B], f32)
            nc.scalar.activation(csb, cp, Act.Identity, bias=cb_sb)

            # ---- per-row d production, one group of lookahead ----
            # fold rows (r2,r3) first: their dp feeds the group's 3rd/5th
            # matmul; PE-style rows' d feeds the 6th/7th.
            dtiles = {}

            def emit_d(g):
                if not (0 <= g < BLK):
                    return
                r0, r1, r2, r3 = rows_of(g)
                for r in (r2, r3):
                    da = dwork.tile([HID, B], f16, name="dtile")
                    nc.vector.tensor_scalar(
                        da, hbf, hT[:, r : r + 1], csb[:, r : r + 1],
                        Alu.min, Alu.add,
                    )
                    dp = dwork.tile([HID, B], f16, name="dtile")
                    nc.vector.tensor_tensor(dp, da, msb, Alu.add)
                    dtiles[r] = dp
                for r in (r0, r1):
                    d = dwork.tile([HID, B], f16, name="dtile")
                    nc.vector.tensor_scalar(
                        d, hbf, hT[:, r : r + 1], csb[:, r : r + 1],
                        Alu.min, Alu.add,
                    )
                    dtiles[r] = d

            # out accumulation: phase p (groups 8p..8p+7) accumulates into its
            # own PSUM bank outp[p]; group g writes partition 32b + (g - 8p) of
            # block b's col_grp strip.  The 4 blocks hit disjoint col_grp
            # strips of the PE array and their out matmuls run concurrently.
            PH = BLK // 2  # 8 groups per phase
            outp = [
                penc.tile([HID, B], f32, name="outp0", tag="encp"),
                penc.tile([HID, B], f32, name="outp1", tag="encp"),
            ]

            def rows_of(g):
                return [g + BLK * b for b in range(NBLK)] if 0 <= g < BLK else []

            pending = {}

            def emit_outs(g):
                p, go = divmod(g, PH)
                for b in range(NBLK):
                    r = g + BLK * b
                    hid_r = pending.pop(r)
                    nc.tensor.matmul(
                        outp[p][32 * b : 32 * b + PH, :],
                        lhsT=embbuf[:, BLK - 1 - go : BLK - 1 - go + PH],
                        rhs=hid_r,
                        start=(go == 0), stop=(go == PH - 1),
                        skip_group_check=True,
                        tile_position=(0, 32 * b),
                    )

            flush_state = {}

            def emit_flush_copy(p, half, eng):
                # copy PSUM -> SBUF in two column-halves (bp2 is added on the
                # host after the gather); ACT has no tensor_copy, so it uses
                # an Identity activation
                if p not in flush_state:
                    flush_state[p] = setup.tile([HID, B], f32, name=f"outs{p}")
                o = flush_state[p]
                sl = slice(half * HB2, (half + 1) * HB2)
                if eng is nc.scalar:
                    eng.activation(o[:, sl], outp[p][:, sl], Act.Identity)
                else:
                    eng.tensor_copy(o[:, sl], outp[p][:, sl])

            def emit_flush_dmas(p):
                # phase-0 descgen on sync/gpsimd (mid-steady; the gpsimd
                # sequencer is idle); phase-1 at the tail on sync/scalar
                # (gpsimd is kept strictly off the tail: measured +9us
                # regression from its SWDGE drain there)
                o = flush_state[p]
                fq = (
                    [nc.sync, nc.gpsimd, nc.sync, nc.gpsimd]
                    if p == 0
                    else [nc.sync, nc.scalar, nc.sync, nc.scalar]
                )
                for b in range(NBLK):
                    fq[b].dma_start(
                        out=out_d[BLK * b + PH * p : BLK * b + PH * (p + 1), :],
                        in_=o[32 * b : 32 * b + PH, :],
                    )

            # prime the d pipeline
            emit_d(0)

            # ---- pairwise main loop: 16 groups of 4 rows ----
            for g in range(BLK):
                r0, r1, r2, r3 = rows_of(g)
                emit_d(g + 1)
                # phase-0 flush copy halves on DVE where it has slack; its
                # last accumulate lands at g=9 (outs of g=7 deferred 2)
                if g == 12:
                    emit_flush_copy(0, 0, nc.vector)
                if g == 13:
                    emit_flush_copy(0, 1, nc.vector)
                    emit_flush_dmas(0)
                duoA = pduo.tile([HID, 2 * B], f32, name="duo")
                duoB = pduo.tile([HID, 2 * B], f32, name="duo")
                nc.tensor.matmul(
                    duoA[:, 0:B], lhsT=w2p_sb, rhs=hbf,
                    start=True, stop=False, skip_group_check=True,
                )
                nc.tensor.matmul(
                    duoA[:, B : 2 * B], lhsT=w2p_sb, rhs=hbf,
                    start=True, stop=False, skip_group_check=True,
                )
                nc.tensor.matmul(
                    duoB[:, 0:B], lhsT=w3_sb, rhs=dtiles.pop(r2),
                    start=True, stop=True, skip_group_check=True,
                )
                if g - DEFER_G >= 0:
                    emit_outs(g - DEFER_G)
                nc.tensor.matmul(
                    duoB[:, B : 2 * B], lhsT=w3_sb, rhs=dtiles.pop(r3),
                    start=True, stop=True, skip_group_check=True,
                )
                nc.tensor.matmul(
                    duoA[:, 0:B], lhsT=w3_sb, rhs=dtiles.pop(r0),
                    start=False, stop=True, skip_group_check=True,
                )
                nc.tensor.matmul(
                    duoA[:, B : 2 * B], lhsT=w3_sb, rhs=dtiles.pop(r1),
                    start=False, stop=True, skip_group_check=True,
                )
                # biasless relus: one ACT op per duo (B first: its matmuls
                # finished earlier)
                hidB = hwork.tile([HID, 2 * B], f16, name="hid")
                nc.scalar.activation(hidB, duoB, Act.Relu)
                hidA = hwork.tile([HID, 2 * B], f16, name="hid")
                nc.scalar.activation(hidA, duoA, Act.Relu)
                pending[r0] = hidA[:, 0:B]
                pending[r1] = hidA[:, B : 2 * B]
                pending[r2] = hidB[:, 0:B]
                pending[r3] = hidB[:, B : 2 * B]
            for g in range(BLK - DEFER_G, BLK):
                emit_outs(g)
            # tail flush: copy halves in parallel on DVE + ACT (both idle)
            emit_flush_copy(1, 0, nc.vector)
            emit_flush_copy(1, 1, nc.scalar)
            emit_flush_dmas(1)

    nc.finalize()
    return nc


def _get_program():
    if "nc" not in _PROGRAM_CACHE:
        _PROGRAM_CACHE["nc"] = _build_program()
    return _PROGRAM_CACHE["nc"]


def _make_in_maps(x, W1, b1, W2, b2, Wp1, bp1, Wp2, bp2):
    f16 = np.float16
    f32 = np.float32
    f64 = np.float64
    x = np.asarray(x, dtype=f32)
    W1 = np.asarray(W1, dtype=f32)
    W2 = np.asarray(W2, dtype=f32)
    Wp1 = np.asarray(Wp1, dtype=f64)
    Wp2 = np.asarray(Wp2, dtype=f32).reshape(HID, 1)
    b1c = np.ascontiguousarray(np.asarray(b1, dtype=f32).reshape(HID, 1))
    b2c = np.ascontiguousarray(np.asarray(b2, dtype=f32).reshape(HID, 1))
    bp1c = np.asarray(bp1, dtype=f64).reshape(HID, 1)

    # |h_i - h_j| = h_i + h_j - 2*min(h_i, h_j) folds (see module docstring)
    w3f = Wp1[2 * HID : 3 * HID, :]
    wp1a = Wp1[0:HID, :] + w3f
    w2p = Wp1[HID : 2 * HID, :] + w3f
    w3 = -2.0 * w3f
    # w3^{-1} folds: M = G1^T h adds the w2p^T h term through the w3 matmul;
    # C = G2^T h + w3^{-T} bp1 turns the per-row relu bias into a per-row
    # per-partition scalar added inside the min's tensor_scalar
    w3i = np.linalg.inv(w3)
    G1 = w2p @ w3i
    G2 = wp1a @ w3i
    cb = (w3i.T @ bp1c).astype(f32)

    NKT = len(KT)
    KPAD = NKT * HID  # 640: in_dim padded so every k-tile is 128 partitions

    # packed weights [w2 | w2p | w3 | G1 | G2], biases [b1 | b2 | cb | wp2]
    wpack = np.concatenate(
        [W2.astype(f64), w2p, w3, G1, G2], axis=1
    ).astype(f16)
    biases = np.zeros((HID, 4), dtype=f32)
    biases[:, 0:1] = b1c
    biases[:, 1:2] = b2c
    biases[:, 2:3] = cb
    biases[:, 3:4] = Wp2

    # w1 padded to [640, 128], viewed as [128, 5*128]
    w1_pad = np.zeros((KPAD, HID), dtype=f32)
    w1_pad[:IN_DIM] = np.asarray(W1, dtype=f32)
    w1p = np.ascontiguousarray(
        w1_pad.reshape(NKT, HID, HID).transpose(1, 0, 2).reshape(HID, NKT * HID)
    ).astype(f16)

    shared = dict(w1p=w1p, wpack=wpack, biases=biases)
    in_maps = []
    for c in range(NCORES):
        xr = np.roll(x, -c * RPC, axis=0)
        xt_pad = np.zeros((KPAD, B), dtype=f32)
        xt_pad[:IN_DIM] = xr.T
        xtp = np.ascontiguousarray(
            xt_pad.reshape(NKT, HID, B).transpose(1, 0, 2).reshape(HID, NKT * B)
        ).astype(f16)
        m = dict(shared)
        m["xtp"] = xtp
        in_maps.append(m)
    return in_maps


def _run(in_maps, trace=False):
    from concourse.bass_utils import run_bass_kernel_spmd

    nc = _get_program()
    return run_bass_kernel_spmd(
        nc, in_maps, core_ids=list(range(NCORES)), trace=trace
    )


def kernel(x, W1, b1, W2, b2, Wp1, bp1, Wp2, bp2):
    in_maps = _make_in_maps(x, W1, b1, W2, b2, Wp1, bp1, Wp2, bp2)
    res = _run(in_maps, trace=False)
    bp2v = np.float32(np.asarray(bp2, dtype=np.float32).reshape(-1)[0])
    out = np.empty((B, B), dtype=np.float32)
    for c in range(NCORES):
        blk = np.asarray(res.results[c]["out"], dtype=np.float32)
        # device block row r*BLK.. maps rows (g + BLK*b); device row order is
        # [g + 16b] = natural order, so rows are already 0..63
        out[c * RPC : (c + 1) * RPC, :] = np.roll(blk, c * RPC, axis=1) + bp2v
    return out


# revision 9
# speedup vs baseline: 1.1831x; 1.1831x over previous
"""Trainium2 Bass kernel for ExemplarGNN2AdjModel (gnn_message_passing).

Math:
  h  = relu(relu(x@W1+b1)@W2+b2)                      # [512,128] node encoder
  scores[i,j] = Wp2 . relu(Wp1a.h_i + Wp1b.h_j + Wp1c.|h_i-h_j| + bp1) + bp2

Device algorithm (per core, SPMD over 8 cores; core c handles 64 rows of i):
  - Each core receives x pre-rolled by c*64 rows and pre-transposed (xT), so the
    identical program computes rows [c*64, c*64+64) in its local (rolled) node
    order; the host un-rolls the output columns afterwards.
  - |h_i-h_j| = h_i + h_j - 2*min(h_i,h_j): the h_i term is folded into the
    per-i bias matrix (wp1a += w3), the h_j term into the B matmul
    (w2p += w3), and the per-pair part is -2*w3^T min(h_i, h_j).
  - w3^{-1} fold (row r2 of each group): M = (w2p w3^{-1})^T h satisfies
    w3^T M = w2p^T h, so P_r = w3^T(min(h,h_r) + M) needs ONE matmul.  The
    fused DVE scalar_tensor_tensor computes (h min h_r) add M in one ~660ns
    pass; this trades 244ns of PE for ~384ns of extra DVE per group and
    balances the engines (PE was the wall).  Only ONE row per group folds:
    the DVE is near-saturated (k=2 measured slower).  M is ~10x larger than
    h, so the whole kernel runs fp16 (10 mantissa bits; same PE/DVE rate as
    bf16) -- measured rel err ~1e-3.  Measured dead ends kept out: GPSIMD
    elementwise (SBUF-port contention inflates concurrent DVE ops ~60%),
    multi-bank PSUM "duo" tiles + batched 1024-col relus (ACT init does not
    amortize: 1197ns vs 2x597; the 3-duo ring stalls the PE), and the
    tensor_scalar (min,add) c-bias fold (+114ns/op on DVE for nothing once
    batching fails).
  - Encoder runs on-device in fp16 (all 512 nodes, replicated per core),
    fp32 PSUM accumulation, fp32 biases.  A2 = wp1a^T h + bp1 and M are
    precomputed once in the ramp.
  - The 64 rows are processed in 16 groups of 4, one row from each of the four
    16-row output blocks (i, i+16, i+32, i+48).  Per group (d tiles produced
    one group ahead; r2's stt first -- it is the DVE's slowest op):
      PE: w2p starts for r0,r1,r3 | w3 r2 (single) | outs g-2 | w3 stops
          r0,r1,r3  (acc pairs 5 slots apart so the same-bank accumulate
          never stalls on the PSUM drain)
      relus: r0,r1,r3 on ACT (bias=A2 col); r2 split ACT[:S]+DVE[S:] to
          balance the engines (ACT 3 full + piece ~= DVE mins+stt+piece).
  - out[16b+i,:] += embW_r^T hid_r: 4 col-tiled PE matmuls to PSUM partitions
    32b..32b+8 of the phase bank -- disjoint col_grp strips run concurrently
    (~1 slot for all 4).  embW_r = embbuf[:, 15-i : 31-i]: a sliding window
    over a 31-column zero buffer with Wp2 at column 15 puts Wp2 exactly in
    stationary column i.  Out matmuls of group g are issued in group g+2 so
    the in-order PE never waits on a relu.  Two 8-group phases accumulate
    into separate banks outp0/outp1 (PH=16 in one bank measured +219ns/group
    of PE on the out matmuls; separate banks also decouple the phase-0 flush
    from phase 1's first accumulate).
  - Output: bp2 is added on the host after the gather.  Phase-0 flush: copy
    halves on DVE in groups 12-13, DMAs on sync/gpsimd mid-steady.  Phase-1
    flush at the tail: copy halves on DVE+ACT in parallel, DMAs on
    sync/scalar (gpsimd is kept strictly off the tail: its SWDGE drain was
    measured at +9us there).
  - Startup: xtp is DMA'd in 5 k-chunks with doorbells spread across the sync/
    gpsimd queues (doorbells cost ~600ns each and serialize per queue);
    encoder matmuls start as chunks land, with narrow [0:128] first-pieces of
    relu1/h2/hbf so the hT/min chain launches early; small dummy matmuls
    bridge every DMA/relu gap so the PE HAM clock-gate stays at 2.4 GHz.
"""

import numpy as np

B = 512
IN_DIM = 595
HID = 128
NCORES = 8
RPC = B // NCORES  # rows per core = 64
NBLK = 4           # output col-tile blocks
BLK = RPC // NBLK  # 16 rows per block
DEFER_G = 2        # groups between producing hid and its out matmul
N_WARM_MM = 8      # dummy matmuls: sustained PE activity trips the HAM
                   # clock-gate to 2.4 GHz before the encoder matmuls run
WARM_N = 256       # free dim of warm matmuls
SPLIT_S = 112      # r2 relu split column: ACT [0:S], DVE [S:512]

# in_dim k-tiles for the first encoder matmul (contraction over 595)
KT = [(0, 128), (128, 256), (256, 384), (384, 512), (512, 595)]

_PROGRAM_CACHE = {}


def _build_program():
    import concourse.mybir as mybir
    import concourse.tile as tile
    from concourse import bacc

    f32 = mybir.dt.float32
    f16 = mybir.dt.float16
    Act = mybir.ActivationFunctionType
    Alu = mybir.AluOpType

    nc = bacc.Bacc("TRN2", target_bir_lowering=False)

    NKT = len(KT)
    xt_d = nc.dram_tensor("xtp", [HID, NKT * B], f16, kind="ExternalInput")
    w1_d = nc.dram_tensor("w1p", [HID, NKT * HID], f16, kind="ExternalInput")
    wpack_d = nc.dram_tensor("wpack", [HID, 5 * HID], f16, kind="ExternalInput")
    bias_d = nc.dram_tensor("biases", [HID, 5], f32, kind="ExternalInput")
    out_d = nc.dram_tensor("out", [RPC, B], f32, kind="ExternalOutput")

    with tile.TileContext(nc) as tc:
        with (
            tc.tile_pool(name="consts", bufs=1) as consts,
            tc.tile_pool(name="setup", bufs=1) as setup,
            tc.tile_pool(name="dwork", bufs=9) as dwork,
            tc.tile_pool(name="hwork", bufs=13) as hwork,
            tc.tile_pool(name="penc", bufs=2, space="PSUM") as penc,
            tc.tile_pool(name="ppair", bufs=6, space="PSUM") as ppair,
        ):
            # ---- input loads first: doorbells cost ~600ns each and serialize
            # per queue, so spread the xtp chunks across three idle queues.
            xt_all = consts.tile([HID, NKT * B], f16)
            w1_all = consts.tile([HID, NKT * HID], f16)
            biases = consts.tile([HID, 5], f32)
            wpack = consts.tile([HID, 5 * HID], f16)
            # earliest-needed first; k-chunks split across sync/gpsimd queues
            nc.scalar.dma_start(out=w1_all, in_=w1_d[:, :])
            qeng = [nc.sync, nc.gpsimd, nc.sync, nc.gpsimd, nc.sync]
            for k in range(NKT):
                qeng[k].dma_start(
                    out=xt_all[:, k * B : (k + 1) * B],
                    in_=xt_d[:, k * B : (k + 1) * B],
                )
            nc.scalar.dma_start(out=biases, in_=bias_d[:, :])
            nc.scalar.dma_start(out=wpack, in_=wpack_d[:, :])

            # ---- PE warm-up over the DMA window (HAM ramps to 2.4 GHz)
            scratch = setup.tile([HID, B], f16)
            nc.vector.memset(scratch, 0.0)
            scratch1 = setup.tile([HID, 1], f32)
            nc.scalar.activation(scratch1, scratch[:, 0:1], Act.Relu)

            def warm_mm(n, w=WARM_N):
                # dummy matmuls keep the PE busy (HAM clock-gate stays at
                # 2.4 GHz) across DMA-wait and relu-wait gaps; they use ppair
                # banks so they never touch the encoder/out accumulator banks
                for _ in range(n):
                    wp = ppair.tile([HID, B], f32, name="pp")
                    nc.tensor.matmul(
                        wp[:, 0:w], lhsT=scratch[:, 0:HID], rhs=scratch[:, 0:w],
                        start=True, stop=True, skip_group_check=True,
                    )

            warm_mm(N_WARM_MM)

            # sliding-window Wp2 buffer: zeros with Wp2 at column BLK-1; the
            # out matmul for block-row i uses embbuf[:, BLK-1-i+c] == Wp2 iff
            # c == i.
            embbuf = consts.tile([HID, 2 * BLK - 1], f16)
            nc.vector.memset(embbuf, 0.0)

            xt_sb = [xt_all[:, k * B : (k + 1) * B] for k in range(NKT)]
            w1_sb = [w1_all[:, k * HID : (k + 1) * HID] for k in range(NKT)]
            w2_sb = wpack[:, 0 * HID : 1 * HID]
            wp1a_sb = wpack[:, 1 * HID : 2 * HID]
            w2p_sb = wpack[:, 2 * HID : 3 * HID]
            w3_sb = wpack[:, 3 * HID : 4 * HID]
            g1_sb = wpack[:, 4 * HID : 5 * HID]
            b1_sb = biases[:, 0:1]
            b2_sb = biases[:, 1:2]
            bp1_sb = biases[:, 2:3]

            # ---- encoder: h1 = relu(W1^T xT + b1), hT = relu(W2^T h1 + b2) ----
            h1p = penc.tile([HID, B], f32, name="encp", tag="encp")
            for k in range(len(KT)):
                nc.tensor.matmul(
                    h1p, lhsT=w1_sb[k], rhs=xt_sb[k],
                    start=(k == 0), stop=(k == len(KT) - 1),
                )
                if k > 0:
                    warm_mm(1)  # bridge the DMA-gated gaps between k-chunks
            # encoder relus split: a narrow ACT first-piece [0:128] lets the
            # h2 -> hbf -> hT -> min chain start early; DVE takes the rest
            HQ = B // 4
            HB2 = B // 2
            h1bf = setup.tile([HID, B], f16)
            nc.scalar.activation(h1bf[:, 0:HQ], h1p[:, 0:HQ], Act.Relu, bias=b1_sb)
            nc.vector.tensor_scalar(
                h1bf[:, HQ:B], h1p[:, HQ:B], b1_sb, 0.0, Alu.add, Alu.max
            )

            # h2 in two matmuls to different PSUM banks: a narrow first piece
            # [0:128] feeding ACT, the rest feeding DVE, so the hT/min chain
            # starts as early as possible
            h2p = penc.tile([HID, HQ], f32, name="encp2", tag="encp")
            nc.tensor.matmul(h2p, lhsT=w2_sb, rhs=h1bf[:, 0:HQ], start=True, stop=True)
            h2pb = ppair.tile([HID, B], f32, name="pp")
            nc.tensor.matmul(
                h2pb[:, 0 : B - HQ], lhsT=w2_sb, rhs=h1bf[:, HQ:B],
                start=True, stop=True, skip_group_check=True,
            )
            warm_mm(2)  # bridge PE over relu2 + hT
            hbf = setup.tile([HID, B], f16)
            nc.scalar.activation(hbf[:, 0:HQ], h2p, Act.Relu, bias=b2_sb)
            nc.vector.tensor_scalar(
                hbf[:, HQ:B], h2pb[:, 0 : B - HQ], b2_sb, 0.0, Alu.add, Alu.max
            )
            # hT fp32 is the per-row scalar operand of the min (tensor_scalar
            # scalars must be fp32); only the core's 64 local-row columns are
            # ever read, and deriving it from hbf avoids a second serialized
            # read of the h2p PSUM bank
            hT = setup.tile([HID, RPC], f32)
            nc.vector.tensor_copy(hT, hbf[:, 0:RPC])
            # Wp2 rides in as f32 column 4 of biases; cast into the sliding
            # window buffer.  Emitted HERE (needed only by the first out
            # matmuls): emitting it earlier head-of-line-blocks the in-order
            # DVE queue on the biases DMA and delays the encoder relus ~1us.
            nc.vector.tensor_copy(embbuf[:, BLK - 1 : BLK], biases[:, 4:5])

            # ---- A2 = wp1a^T h + bp1  (per-i relu bias columns) ----
            a2p = penc.tile([HID, B], f32, name="encp3", tag="encp")
            nc.tensor.matmul(a2p, lhsT=wp1a_sb, rhs=hbf, start=True, stop=True)
            warm_mm(1)
            # ---- M = (w2p w3^{-1})^T h, PSUM -> SBUF fp16 ----
            mp = penc.tile([HID, B], f32, name="encp4", tag="encp")
            nc.tensor.matmul(mp, lhsT=g1_sb, rhs=hbf, start=True, stop=True)
            warm_mm(1)  # bridge PE over the first mins
            a2 = setup.tile([HID, B], f32)
            nc.scalar.activation(a2, a2p, Act.Identity, bias=bp1_sb)
            msb = setup.tile([HID, B], f16)
            nc.vector.tensor_copy(msb, mp)

            # ---- per-row d production, one group of lookahead ----
            # r2's fused stt first: it is the DVE's slowest op and feeds the
            # group's 4th matmul; plain mins feed the 6th-8th.
            dtiles = {}

            def emit_d(g):
                if not (0 <= g < BLK):
                    return
                r0, r1, r2, r3 = rows_of(g)
                dp = dwork.tile([HID, B], f16, name="dtile")
                nc.vector.scalar_tensor_tensor(
                    dp, hbf, hT[:, r2 : r2 + 1], msb, Alu.min, Alu.add
                )
                dtiles[r2] = dp
                for r in (r0, r1, r3):
                    d = dwork.tile([HID, B], f16, name="dtile")
                    nc.vector.tensor_scalar(
                        d, hbf, hT[:, r : r + 1], None, Alu.min
                    )
                    dtiles[r] = d

            # out accumulation: phase p (groups 8p..8p+7) accumulates into its
            # own PSUM bank outp[p]; group g writes partition 32b + (g - 8p) of
            # block b's col_grp strip.  The 4 blocks hit disjoint col_grp
            # strips of the PE array and their out matmuls run concurrently.
            PH = BLK // 2  # 8 groups per phase
            outp = [
                penc.tile([HID, B], f32, name="outp0", tag="encp"),
                penc.tile([HID, B], f32, name="outp1", tag="encp"),
            ]

            def rows_of(g):
                return [g + BLK * b for b in range(NBLK)] if 0 <= g < BLK else []

            pending = {}

            def emit_outs(g):
                p, go = divmod(g, PH)
                for b in range(NBLK):
                    r = g + BLK * b
                    hid_r = pending.pop(r)
                    nc.tensor.matmul(
                        outp[p][32 * b : 32 * b + PH, :],
                        lhsT=embbuf[:, BLK - 1 - go : BLK - 1 - go + PH],
                        rhs=hid_r,
                        start=(go == 0), stop=(go == PH - 1),
                        skip_group_check=True,
                        tile_position=(0, 32 * b),
                    )

            flush_state = {}

            def emit_flush_copy(p, half, eng):
                # copy PSUM -> SBUF in two column-halves (bp2 is added on the
                # host after the gather); ACT has no tensor_copy, so it uses
                # an Identity activation
                if p not in flush_state:
                    flush_state[p] = setup.tile([HID, B], f32, name=f"outs{p}")
                o = flush_state[p]
                sl = slice(half * HB2, (half + 1) * HB2)
                if eng is nc.scalar:
                    eng.activation(o[:, sl], outp[p][:, sl], Act.Identity)
                else:
                    eng.tensor_copy(o[:, sl], outp[p][:, sl])

            def emit_flush_dmas(p):
                # phase-0 descgen on sync/gpsimd (mid-steady; the gpsimd
                # sequencer is idle); phase-1 at the tail on sync/scalar
                # (gpsimd is kept strictly off the tail: measured +9us
                # regression from its SWDGE drain there)
                o = flush_state[p]
                fq = (
                    [nc.sync, nc.gpsimd, nc.sync, nc.gpsimd]
                    if p == 0
                    else [nc.sync, nc.scalar, nc.sync, nc.scalar]
                )
                for b in range(NBLK):
                    fq[b].dma_start(
                        out=out_d[BLK * b + PH * p : BLK * b + PH * (p + 1), :],
                        in_=o[32 * b : 32 * b + PH, :],
                    )

            # prime the d pipeline
            emit_d(0)

            # ---- pairwise main loop: 16 groups of 4 rows ----
            for g in range(BLK):
                r0, r1, r2, r3 = rows_of(g)
                emit_d(g + 1)
                # phase-0 flush copy halves on DVE; its last accumulate lands
                # at g=9 (outs of g=7 deferred 2)
                if g == 12:
                    emit_flush_copy(0, 0, nc.vector)
                if g == 13:
                    emit_flush_copy(0, 1, nc.vector)
                    emit_flush_dmas(0)
                pps = {}
                for r in (r0, r1, r3):
                    pp = ppair.tile([HID, B], f32, name="pp")
                    nc.tensor.matmul(
                        pp, lhsT=w2p_sb, rhs=hbf,
                        start=True, stop=False, skip_group_check=True,
                    )
                    pps[r] = pp
                pp2 = ppair.tile([HID, B], f32, name="pp")
                nc.tensor.matmul(
                    pp2, lhsT=w3_sb, rhs=dtiles.pop(r2),
                    start=True, stop=True, skip_group_check=True,
                )
                pps[r2] = pp2
                if g - DEFER_G >= 0:
                    emit_outs(g - DEFER_G)
                for r in (r0, r1, r3):
                    nc.tensor.matmul(
                        pps[r], lhsT=w3_sb, rhs=dtiles.pop(r),
                        start=False, stop=True, skip_group_check=True,
                    )
                # relus: r2 split ACT[0:S] + DVE[S:], r0/r1/r3 full on ACT.
                # r2's pieces can issue early (its matmul is the group's 4th
                # slot); in the last group there is no next-group d work, so
                # give the DVE a full relu to shorten the ACT chain.
                hid2 = hwork.tile([HID, B], f16, name="hid")
                if g == BLK - 1:
                    nc.vector.tensor_scalar(
                        hid2, pp2, a2[:, r2 : r2 + 1], 0.0, Alu.add, Alu.max
                    )
                else:
                    nc.scalar.activation(
                        hid2[:, 0:SPLIT_S], pp2[:, 0:SPLIT_S], Act.Relu,
                        bias=a2[:, r2 : r2 + 1],
                    )
                    nc.vector.tensor_scalar(
                        hid2[:, SPLIT_S:B], pp2[:, SPLIT_S:B],
                        a2[:, r2 : r2 + 1], 0.0, Alu.add, Alu.max,
                    )
                pending[r2] = hid2
                for r in (r0, r1, r3):
                    hid = hwork.tile([HID, B], f16, name="hid")
                    nc.scalar.activation(
                        hid, pps[r], Act.Relu, bias=a2[:, r : r + 1]
                    )
                    pending[r] = hid
            for g in range(BLK - DEFER_G, BLK):
                emit_outs(g)
            # tail flush: copy halves in parallel on DVE + ACT (both idle)
            emit_flush_copy(1, 0, nc.vector)
            emit_flush_copy(1, 1, nc.scalar)
            emit_flush_dmas(1)

    nc.finalize()
    return nc


def _get_program():
    if "nc" not in _PROGRAM_CACHE:
        _PROGRAM_CACHE["nc"] = _build_program()
    return _PROGRAM_CACHE["nc"]


def _make_in_maps(x, W1, b1, W2, b2, Wp1, bp1, Wp2, bp2):
    f16 = np.float16
    f32 = np.float32
    f64 = np.float64
    x = np.asarray(x, dtype=f32)
    W1 = np.asarray(W1, dtype=f32)
    W2 = np.asarray(W2, dtype=f32)
    Wp1 = np.asarray(Wp1, dtype=f64)
    Wp2 = np.asarray(Wp2, dtype=f32).reshape(HID, 1)
    b1c = np.ascontiguousarray(np.asarray(b1, dtype=f32).reshape(HID, 1))
    b2c = np.ascontiguousarray(np.asarray(b2, dtype=f32).reshape(HID, 1))
    bp1c = np.ascontiguousarray(np.asarray(bp1, dtype=f32).reshape(HID, 1))

    # |h_i - h_j| = h_i + h_j - 2*min(h_i, h_j) folds (see module docstring)
    w3f = Wp1[2 * HID : 3 * HID, :]
    wp1a = Wp1[0:HID, :] + w3f
    w2p = Wp1[HID : 2 * HID, :] + w3f
    w3 = -2.0 * w3f
    # w3^{-1} fold: M = G1^T h adds the w2p^T h term through the w3 matmul
    G1 = w2p @ np.linalg.inv(w3)

    NKT = len(KT)
    KPAD = NKT * HID  # 640: in_dim padded so every k-tile is 128 partitions

    # packed weights [w2 | wp1a | w2p | w3 | G1],
    # biases [b1 | b2 | bp1 | bp2 | wp2] (bp2 unused on device)
    wpack = np.concatenate(
        [W2.astype(f64), wp1a, w2p, w3, G1], axis=1
    ).astype(f16)
    biases = np.zeros((HID, 5), dtype=f32)
    biases[:, 0:1] = b1c
    biases[:, 1:2] = b2c
    biases[:, 2:3] = bp1c
    biases[:, 4:5] = Wp2

    # w1 padded to [640, 128], viewed as [128, 5*128]
    w1_pad = np.zeros((KPAD, HID), dtype=f32)
    w1_pad[:IN_DIM] = np.asarray(W1, dtype=f32)
    w1p = np.ascontiguousarray(
        w1_pad.reshape(NKT, HID, HID).transpose(1, 0, 2).reshape(HID, NKT * HID)
    ).astype(f16)

    shared = dict(w1p=w1p, wpack=wpack, biases=biases)
    in_maps = []
    for c in range(NCORES):
        xr = np.roll(x, -c * RPC, axis=0)
        xt_pad = np.zeros((KPAD, B), dtype=f32)
        xt_pad[:IN_DIM] = xr.T
        xtp = np.ascontiguousarray(
            xt_pad.reshape(NKT, HID, B).transpose(1, 0, 2).reshape(HID, NKT * B)
        ).astype(f16)
        m = dict(shared)
        m["xtp"] = xtp
        in_maps.append(m)
    return in_maps


def _run(in_maps, trace=False):
    from concourse.bass_utils import run_bass_kernel_spmd

    nc = _get_program()
    return run_bass_kernel_spmd(
        nc, in_maps, core_ids=list(range(NCORES)), trace=trace
    )


def kernel(x, W1, b1, W2, b2, Wp1, bp1, Wp2, bp2):
    in_maps = _make_in_maps(x, W1, b1, W2, b2, Wp1, bp1, Wp2, bp2)
    res = _run(in_maps, trace=False)
    bp2v = np.float32(np.asarray(bp2, dtype=np.float32).reshape(-1)[0])
    out = np.empty((B, B), dtype=np.float32)
    for c in range(NCORES):
        blk = np.asarray(res.results[c]["out"], dtype=np.float32)
        # device block row r*BLK.. maps rows (g + BLK*b); device row order is
        # [g + 16b] = natural order, so rows are already 0..63
        out[c * RPC : (c + 1) * RPC, :] = np.roll(blk, c * RPC, axis=1) + bp2v
    return out


# revision 13
# speedup vs baseline: 1.2588x; 1.0641x over previous
"""Trainium2 Bass kernel for ExemplarGNN2AdjModel (gnn_message_passing).

Math:
  h  = relu(relu(x@W1+b1)@W2+b2)                      # [512,128] node encoder
  scores[i,j] = Wp2 . relu(Wp1a.h_i + Wp1b.h_j + Wp1c.|h_i-h_j| + bp1) + bp2

Device algorithm (per core, SPMD over 8 cores; core c handles 64 rows of i):
  - Each core receives x pre-rolled by c*64 rows and pre-transposed (xT), so the
    identical program computes rows [c*64, c*64+64) in its local (rolled) node
    order; the host un-rolls the output columns afterwards.
  - |h_i-h_j| = h_i + h_j - 2*min(h_i,h_j): the h_i term is folded into the
    per-i bias matrix (wp1a += w3), the h_j term into the B matmul
    (w2p += w3), and the per-pair part is -2*w3^T min(h_i, h_j).
  - w3^{-1} fold (row r2 of each group): M = (w2p w3^{-1})^T h satisfies
    w3^T M = w2p^T h, so P_r = w3^T(min(h,h_r) + M) needs ONE matmul.  The
    fused DVE scalar_tensor_tensor computes (h min h_r) add M in one ~660ns
    pass; this trades 244ns of PE for ~384ns of extra DVE per group and
    balances the engines (PE was the wall).  Only ONE row per group folds:
    the DVE is near-saturated (k=2 measured slower).  M is ~10x larger than
    h, so the whole kernel runs fp16 (10 mantissa bits; same PE/DVE rate as
    bf16) -- measured rel err ~1e-3.  Measured dead ends kept out: GPSIMD
    elementwise (SBUF-port contention inflates concurrent DVE ops ~60%),
    multi-bank PSUM "duo" tiles + batched 1024-col relus (ACT init does not
    amortize: 1197ns vs 2x597; the 3-duo ring stalls the PE), and the
    tensor_scalar (min,add) c-bias fold (+114ns/op on DVE for nothing once
    batching fails).
  - Encoder runs on-device in fp16 (all 512 nodes, replicated per core),
    fp32 PSUM accumulation, fp32 biases.  A2 = wp1a^T h + bp1 and M are
    precomputed once in the ramp.
  - The 64 rows are processed in 16 groups of 4, one row from each of the four
    16-row output blocks (i, i+16, i+32, i+48).  Per group (d tiles produced
    one group ahead; r2's stt first -- it is the DVE's slowest op):
      PE: w2p starts for r0,r1,r3 | w3 r2 (single) | outs g-2 | w3 stops
          r0,r1,r3  (acc pairs 5 slots apart so the same-bank accumulate
          never stalls on the PSUM drain)
      relus: r0,r1,r3 on ACT (bias=A2 col); r2 split ACT[:S]+DVE[S:] to
          balance the engines (ACT 3 full + piece ~= DVE mins+stt+piece).
  - out[16b+i,:] += embW_r^T hid_r: 4 col-tiled PE matmuls to PSUM partitions
    32b..32b+8 of the phase bank -- disjoint col_grp strips run concurrently
    (~1 slot for all 4).  embW_r = embbuf[:, 15-i : 31-i]: a sliding window
    over a 31-column zero buffer with Wp2 at column 15 puts Wp2 exactly in
    stationary column i.  Out matmuls of group g are issued in group g+2 so
    the in-order PE never waits on a relu.  Two 8-group phases accumulate
    into separate banks outp0/outp1 (PH=16 in one bank measured +219ns/group
    of PE on the out matmuls; separate banks also decouple the phase-0 flush
    from phase 1's first accumulate).
  - Output: bp2 is added on the host after the gather.  Phase-0 flush: copy
    halves on DVE in groups 12-13, DMAs on sync/gpsimd mid-steady.  Phase-1
    flush at the tail: copy halves on DVE+ACT in parallel, DMAs on
    sync/scalar (gpsimd is kept strictly off the tail: its SWDGE drain was
    measured at +9us there).
  - Startup: xtp is DMA'd in 5 k-chunks with doorbells spread across the sync/
    gpsimd queues (doorbells cost ~600ns each and serialize per queue);
    encoder matmuls start as chunks land, with narrow [0:128] first-pieces of
    relu1/h2/hbf so the hT/min chain launches early; small dummy matmuls
    bridge every DMA/relu gap so the PE HAM clock-gate stays at 2.4 GHz.
"""

import numpy as np

B = 512
IN_DIM = 595
HID = 128
NCORES = 8
RPC = B // NCORES  # rows per core = 64
NBLK = 4           # output col-tile blocks
BLK = RPC // NBLK  # 16 rows per block
DEFER_G = 2        # groups between producing hid and its out matmul
N_WARM_MM = 8      # dummy matmuls: sustained PE activity trips the HAM
                   # clock-gate to 2.4 GHz before the encoder matmuls run
WARM_N = 256       # free dim of warm matmuls
SPLIT_S = 112      # r2 relu split column: ACT [0:S], DVE [S:512]

# in_dim k-tiles for the first encoder matmul (contraction over 595)
KT = [(0, 128), (128, 256), (256, 384), (384, 512), (512, 595)]

_PROGRAM_CACHE = {}


def _build_program():
    import concourse.mybir as mybir
    import concourse.tile as tile
    from concourse import bacc

    f32 = mybir.dt.float32
    f16 = mybir.dt.float16
    Act = mybir.ActivationFunctionType
    Alu = mybir.AluOpType

    nc = bacc.Bacc("TRN2", target_bir_lowering=False)

    NKT = len(KT)
    xt_d = nc.dram_tensor("xtp", [HID, NKT * B], f16, kind="ExternalInput")
    w1_d = nc.dram_tensor("w1p", [HID, NKT * HID], f16, kind="ExternalInput")
    wpack_d = nc.dram_tensor("wpack", [HID, 5 * HID], f16, kind="ExternalInput")
    bias_d = nc.dram_tensor("biases", [HID, 5], f32, kind="ExternalInput")
    out_d = nc.dram_tensor("out", [RPC, B], f32, kind="ExternalOutput")

    with tile.TileContext(nc) as tc:
        with (
            tc.tile_pool(name="consts", bufs=1) as consts,
            tc.tile_pool(name="setup", bufs=1) as setup,
            tc.tile_pool(name="dwork", bufs=9) as dwork,
            tc.tile_pool(name="hwork", bufs=13) as hwork,
            tc.tile_pool(name="penc", bufs=1, space="PSUM") as penc,
            tc.tile_pool(name="ppair", bufs=7, space="PSUM") as ppair,
        ):
            # ---- input loads first: doorbells cost ~600ns each and serialize
            # per queue, so spread the xtp chunks across three idle queues.
            xt_all = consts.tile([HID, NKT * B], f16)
            w1_all = consts.tile([HID, NKT * HID], f16)
            biases = consts.tile([HID, 5], f32)
            wpack = consts.tile([HID, 5 * HID], f16)
            # earliest-needed first; k-chunks split across sync/gpsimd queues
            nc.scalar.dma_start(out=w1_all, in_=w1_d[:, :])
            qeng = [nc.sync, nc.gpsimd, nc.sync, nc.gpsimd, nc.sync]
            for k in range(NKT):
                qeng[k].dma_start(
                    out=xt_all[:, k * B : (k + 1) * B],
                    in_=xt_d[:, k * B : (k + 1) * B],
                )
            nc.scalar.dma_start(out=biases, in_=bias_d[:, :])
            nc.scalar.dma_start(out=wpack, in_=wpack_d[:, :])

            # ---- PE warm-up over the DMA window (HAM ramps to 2.4 GHz)
            scratch = setup.tile([HID, B], f16)
            nc.vector.memset(scratch, 0.0)
            scratch1 = setup.tile([HID, 1], f32)
            nc.scalar.activation(scratch1, scratch[:, 0:1], Act.Relu)

            def warm_mm(n, w=WARM_N):
                # dummy matmuls keep the PE busy (HAM clock-gate stays at
                # 2.4 GHz) across DMA-wait and relu-wait gaps; they use ppair
                # banks so they never touch the encoder/out accumulator banks
                for _ in range(n):
                    wp = ppair.tile([HID, B], f32, name="pp")
                    nc.tensor.matmul(
                        wp[:, 0:w], lhsT=scratch[:, 0:HID], rhs=scratch[:, 0:w],
                        start=True, stop=True, skip_group_check=True,
                    )

            warm_mm(N_WARM_MM)

            # sliding-window Wp2 buffer: zeros with Wp2 at column BLK-1; the
            # out matmul for block-row i uses embbuf[:, BLK-1-i+c] == Wp2 iff
            # c == i.
            embbuf = consts.tile([HID, 2 * BLK - 1], f16)
            nc.vector.memset(embbuf, 0.0)

            xt_sb = [xt_all[:, k * B : (k + 1) * B] for k in range(NKT)]
            w1_sb = [w1_all[:, k * HID : (k + 1) * HID] for k in range(NKT)]
            w2_sb = wpack[:, 0 * HID : 1 * HID]
            wp1a_sb = wpack[:, 1 * HID : 2 * HID]
            w2p_sb = wpack[:, 2 * HID : 3 * HID]
            w3_sb = wpack[:, 3 * HID : 4 * HID]
            g1_sb = wpack[:, 4 * HID : 5 * HID]
            b1_sb = biases[:, 0:1]
            b2_sb = biases[:, 1:2]
            bp1_sb = biases[:, 2:3]

            # ---- encoder: h1 = relu(W1^T xT + b1), hT = relu(W2^T h1 + b2) ----
            h1p = penc.tile([HID, B], f32, name="encp", tag="encp")
            for k in range(len(KT)):
                nc.tensor.matmul(
                    h1p, lhsT=w1_sb[k], rhs=xt_sb[k],
                    start=(k == 0), stop=(k == len(KT) - 1),
                )
                if k > 0:
                    warm_mm(1)  # bridge the DMA-gated gaps between k-chunks
            # encoder relus split: a narrow ACT first-piece [0:128] lets the
            # h2 -> hbf -> hT -> min chain start early; DVE takes the rest
            HQ = B // 4
            HB2 = B // 2
            h1bf = setup.tile([HID, B], f16)
            nc.scalar.activation(h1bf[:, 0:HQ], h1p[:, 0:HQ], Act.Relu, bias=b1_sb)
            nc.vector.tensor_scalar(
                h1bf[:, HQ:B], h1p[:, HQ:B], b1_sb, 0.0, Alu.add, Alu.max
            )

            # h2 in two matmuls to different PSUM banks: a narrow first piece
            # [0:128] feeding ACT, the rest feeding DVE, so the hT/min chain
            # starts as early as possible
            h2p = penc.tile([HID, HQ], f32, name="encp2", tag="encp")
            nc.tensor.matmul(h2p, lhsT=w2_sb, rhs=h1bf[:, 0:HQ], start=True, stop=True)
            h2pb = ppair.tile([HID, B], f32, name="pp")
            nc.tensor.matmul(
                h2pb[:, 0 : B - HQ], lhsT=w2_sb, rhs=h1bf[:, HQ:B],
                start=True, stop=True, skip_group_check=True,
            )
            warm_mm(2)  # bridge PE over relu2 + hT
            hbf = setup.tile([HID, B], f16)
            nc.scalar.activation(hbf[:, 0:HQ], h2p, Act.Relu, bias=b2_sb)
            nc.vector.tensor_scalar(
                hbf[:, HQ:B], h2pb[:, 0 : B - HQ], b2_sb, 0.0, Alu.add, Alu.max
            )
            # hT fp32 is the per-row scalar operand of the min (tensor_scalar
            # scalars must be fp32); only the core's 64 local-row columns are
            # ever read, and deriving it from hbf avoids a second serialized
            # read of the h2p PSUM bank
            hT = setup.tile([HID, RPC], f32)
            nc.vector.tensor_copy(hT, hbf[:, 0:RPC])
            # Wp2 rides in as f32 column 4 of biases; cast into the sliding
            # window buffer.  Emitted HERE (needed only by the first out
            # matmuls): emitting it earlier head-of-line-blocks the in-order
            # DVE queue on the biases DMA and delays the encoder relus ~1us.
            nc.vector.tensor_copy(embbuf[:, BLK - 1 : BLK], biases[:, 4:5])

            # ---- A2 = wp1a^T h + bp1  (per-i relu bias columns) ----
            a2p = penc.tile([HID, B], f32, name="encp3", tag="encp")
            nc.tensor.matmul(a2p, lhsT=wp1a_sb, rhs=hbf, start=True, stop=True)
            warm_mm(1)
            # ---- M = (w2p w3^{-1})^T h, PSUM -> SBUF fp16; mp is a
            # transient ppair tile so the penc bank stays free for outp
            mp = ppair.tile([HID, B], f32, name="pp")
            nc.tensor.matmul(
                mp, lhsT=g1_sb, rhs=hbf,
                start=True, stop=True, skip_group_check=True,
            )
            warm_mm(1)  # bridge PE over the first mins
            a2 = setup.tile([HID, B], f32)
            nc.scalar.activation(a2, a2p, Act.Identity, bias=bp1_sb)
            msb = setup.tile([HID, B], f16)
            nc.vector.tensor_copy(msb, mp)

            # ---- per-row d production, one group of lookahead ----
            # r2's fused stt first: it is the DVE's slowest op and feeds the
            # group's 4th matmul; plain mins feed the 6th-8th.
            dtiles = {}

            def emit_d(g):
                if not (0 <= g < BLK):
                    return
                r0, r1, r2, r3 = rows_of(g)
                dp = dwork.tile([HID, B], f16, name="dtile")
                nc.vector.scalar_tensor_tensor(
                    dp, hbf, hT[:, r2 : r2 + 1], msb, Alu.min, Alu.add
                )
                dtiles[r2] = dp
                for r in (r0, r1, r3):
                    d = dwork.tile([HID, B], f16, name="dtile")
                    nc.vector.tensor_scalar(
                        d, hbf, hT[:, r : r + 1], None, Alu.min
                    )
                    dtiles[r] = d

            # out accumulation in two phases of 8 groups each, reusing ONE
            # PSUM bank (the encoder bank): phase p group g writes partition
            # 32b + (g - 8p) of block b's col_grp strip; the phase-0 flush
            # (copy to SBUF, 4 strip DMAs) overlaps the phase-1 compute.  The
            # 4 blocks hit disjoint col_grp strips of the PE array and their
            # out matmuls run concurrently.
            PH = BLK // 2  # 8 groups per phase
            outp = penc.tile([HID, B], f32, name="outp", tag="encp")

            def rows_of(g):
                return [g + BLK * b for b in range(NBLK)] if 0 <= g < BLK else []

            pending = {}

            def emit_outs(g):
                go = g % PH
                for b in range(NBLK):
                    r = g + BLK * b
                    hid_r = pending.pop(r)
                    nc.tensor.matmul(
                        outp[32 * b : 32 * b + PH, :],
                        lhsT=embbuf[:, BLK - 1 - go : BLK - 1 - go + PH],
                        rhs=hid_r,
                        start=(go == 0), stop=(go == PH - 1),
                        skip_group_check=True,
                        tile_position=(0, 32 * b),
                    )

            flush_state = {}

            def emit_flush_copy(p, half, eng):
                # copy PSUM -> SBUF in two column-halves (bp2 is added on the
                # host after the gather); ACT has no tensor_copy, so it uses
                # an Identity activation
                if p not in flush_state:
                    flush_state[p] = setup.tile([HID, B], f32, name=f"outs{p}")
                o = flush_state[p]
                sl = slice(half * HB2, (half + 1) * HB2)
                if eng is nc.scalar:
                    eng.activation(o[:, sl], outp[:, sl], Act.Identity)
                else:
                    eng.tensor_copy(o[:, sl], outp[:, sl])

            def emit_flush_dmas(p):
                # phase-0 descgen on sync/gpsimd (mid-steady; the gpsimd
                # sequencer is idle); phase-1 at the tail on sync/scalar
                # (gpsimd is kept strictly off the tail: measured +9us
                # regression from its SWDGE drain there)
                o = flush_state[p]
                fq = (
                    [nc.sync, nc.gpsimd, nc.sync, nc.gpsimd]
                    if p == 0
                    else [nc.sync, nc.scalar, nc.sync, nc.scalar]
                )
                for b in range(NBLK):
                    fq[b].dma_start(
                        out=out_d[BLK * b + PH * p : BLK * b + PH * (p + 1), :],
                        in_=o[32 * b : 32 * b + PH, :],
                    )

            # prime the d pipeline
            emit_d(0)

            # ---- pairwise main loop: 16 groups of 4 rows ----
            for g in range(BLK):
                r0, r1, r2, r3 = rows_of(g)
                emit_d(g + 1)
                # The second flush-copy half must precede phase-1's first out
                # matmuls (start=True clears the shared bank).
                if g - DEFER_G == PH:
                    emit_flush_copy(0, 1, nc.vector)
                    emit_flush_dmas(0)
                pps = {}
                for r in (r0, r1, r3):
                    pp = ppair.tile([HID, B], f32, name="pp")
                    nc.tensor.matmul(
                        pp, lhsT=w2p_sb, rhs=hbf,
                        start=True, stop=False, skip_group_check=True,
                    )
                    pps[r] = pp
                # deferred out matmuls (4 col-tiled, concurrent) sit between
                # the w2p starts and the w3 block: their post-drain penalty
                # overlaps r2's stt wait, the w3 stationary is loaded once for
                # all 4 w3 matmuls, and the same-bank acc pairs get 5 slots of
                # spacing
                if g - DEFER_G >= 0:
                    emit_outs(g - DEFER_G)
                    if g - DEFER_G == PH - 1:
                        emit_flush_copy(0, 0, nc.vector)
                pp2 = ppair.tile([HID, B], f32, name="pp")
                nc.tensor.matmul(
                    pp2, lhsT=w3_sb, rhs=dtiles.pop(r2),
                    start=True, stop=True, skip_group_check=True,
                )
                pps[r2] = pp2
                for r in (r0, r1, r3):
                    nc.tensor.matmul(
                        pps[r], lhsT=w3_sb, rhs=dtiles.pop(r),
                        start=False, stop=True, skip_group_check=True,
                    )
                # relus: r2 split ACT[0:S] + DVE[S:], r0/r1/r3 full on ACT.
                # r2's pieces can issue early (its matmul is the group's 4th
                # slot); in the last group there is no next-group d work, so
                # give the DVE a full relu to shorten the ACT chain.
                hid2 = hwork.tile([HID, B], f16, name="hid")
                if g == BLK - 1:
                    nc.vector.tensor_scalar(
                        hid2, pp2, a2[:, r2 : r2 + 1], 0.0, Alu.add, Alu.max
                    )
                else:
                    nc.scalar.activation(
                        hid2[:, 0:SPLIT_S], pp2[:, 0:SPLIT_S], Act.Relu,
                        bias=a2[:, r2 : r2 + 1],
                    )
                    nc.vector.tensor_scalar(
                        hid2[:, SPLIT_S:B], pp2[:, SPLIT_S:B],
                        a2[:, r2 : r2 + 1], 0.0, Alu.add, Alu.max,
                    )
                pending[r2] = hid2
                for r in (r0, r1, r3):
                    hid = hwork.tile([HID, B], f16, name="hid")
                    nc.scalar.activation(
                        hid, pps[r], Act.Relu, bias=a2[:, r : r + 1]
                    )
                    pending[r] = hid
            for g in range(BLK - DEFER_G, BLK):
                emit_outs(g)
            # tail flush: copy halves in parallel on DVE + ACT (both idle)
            emit_flush_copy(1, 0, nc.vector)
            emit_flush_copy(1, 1, nc.scalar)
            emit_flush_dmas(1)

    nc.finalize()
    return nc


def _get_program():
    if "nc" not in _PROGRAM_CACHE:
        _PROGRAM_CACHE["nc"] = _build_program()
    return _PROGRAM_CACHE["nc"]


def _make_in_maps(x, W1, b1, W2, b2, Wp1, bp1, Wp2, bp2):
    f16 = np.float16
    f32 = np.float32
    f64 = np.float64
    x = np.asarray(x, dtype=f32)
    W1 = np.asarray(W1, dtype=f32)
    W2 = np.asarray(W2, dtype=f32)
    Wp1 = np.asarray(Wp1, dtype=f64)
    Wp2 = np.asarray(Wp2, dtype=f32).reshape(HID, 1)
    b1c = np.ascontiguousarray(np.asarray(b1, dtype=f32).reshape(HID, 1))
    b2c = np.ascontiguousarray(np.asarray(b2, dtype=f32).reshape(HID, 1))
    bp1c = np.ascontiguousarray(np.asarray(bp1, dtype=f32).reshape(HID, 1))

    # |h_i - h_j| = h_i + h_j - 2*min(h_i, h_j) folds (see module docstring)
    w3f = Wp1[2 * HID : 3 * HID, :]
    wp1a = Wp1[0:HID, :] + w3f
    w2p = Wp1[HID : 2 * HID, :] + w3f
    w3 = -2.0 * w3f
    # w3^{-1} fold: M = G1^T h adds the w2p^T h term through the w3 matmul
    G1 = w2p @ np.linalg.inv(w3)

    NKT = len(KT)
    KPAD = NKT * HID  # 640: in_dim padded so every k-tile is 128 partitions

    # packed weights [w2 | wp1a | w2p | w3 | G1],
    # biases [b1 | b2 | bp1 | bp2 | wp2] (bp2 unused on device)
    wpack = np.concatenate(
        [W2.astype(f64), wp1a, w2p, w3, G1], axis=1
    ).astype(f16)
    biases = np.zeros((HID, 5), dtype=f32)
    biases[:, 0:1] = b1c
    biases[:, 1:2] = b2c
    biases[:, 2:3] = bp1c
    biases[:, 4:5] = Wp2

    # w1 padded to [640, 128], viewed as [128, 5*128]
    w1_pad = np.zeros((KPAD, HID), dtype=f32)
    w1_pad[:IN_DIM] = np.asarray(W1, dtype=f32)
    w1p = np.ascontiguousarray(
        w1_pad.reshape(NKT, HID, HID).transpose(1, 0, 2).reshape(HID, NKT * HID)
    ).astype(f16)

    shared = dict(w1p=w1p, wpack=wpack, biases=biases)
    in_maps = []
    for c in range(NCORES):
        xr = np.roll(x, -c * RPC, axis=0)
        xt_pad = np.zeros((KPAD, B), dtype=f32)
        xt_pad[:IN_DIM] = xr.T
        xtp = np.ascontiguousarray(
            xt_pad.reshape(NKT, HID, B).transpose(1, 0, 2).reshape(HID, NKT * B)
        ).astype(f16)
        m = dict(shared)
        m["xtp"] = xtp
        in_maps.append(m)
    return in_maps


def _run(in_maps, trace=False):
    from concourse.bass_utils import run_bass_kernel_spmd

    nc = _get_program()
    return run_bass_kernel_spmd(
        nc, in_maps, core_ids=list(range(NCORES)), trace=trace
    )


def kernel(x, W1, b1, W2, b2, Wp1, bp1, Wp2, bp2):
    in_maps = _make_in_maps(x, W1, b1, W2, b2, Wp1, bp1, Wp2, bp2)
    res = _run(in_maps, trace=False)
    bp2v = np.float32(np.asarray(bp2, dtype=np.float32).reshape(-1)[0])
    out = np.empty((B, B), dtype=np.float32)
    for c in range(NCORES):
        blk = np.asarray(res.results[c]["out"], dtype=np.float32)
        # device block row r*BLK.. maps rows (g + BLK*b); device row order is
        # [g + 16b] = natural order, so rows are already 0..63
        out[c * RPC : (c + 1) * RPC, :] = np.roll(blk, c * RPC, axis=1) + bp2v
    return out


# revision 19
# speedup vs baseline: 1.2834x; 1.0195x over previous
"""Trainium2 Bass kernel for ExemplarGNN2AdjModel (gnn_message_passing).

Math:
  h  = relu(relu(x@W1+b1)@W2+b2)                      # [512,128] node encoder
  scores[i,j] = Wp2 . relu(Wp1a.h_i + Wp1b.h_j + Wp1c.|h_i-h_j| + bp1) + bp2

Device algorithm (per core, SPMD over 8 cores; core c handles 64 rows of i):
  - Each core receives x pre-rolled by c*64 rows and pre-transposed (xT), so the
    identical program computes rows [c*64, c*64+64) in its local (rolled) node
    order; the host un-rolls the output columns afterwards.
  - |h_i-h_j| = h_i + h_j - 2*min(h_i,h_j): the h_i term is folded into the
    per-i bias matrix (wp1a += w3), the h_j term into the B matmul
    (w2p += w3), and the per-pair part is -2*w3^T min(h_i, h_j).
  - w3^{-1} fold (row r2 of each group): M = (w2p w3^{-1})^T h satisfies
    w3^T M = w2p^T h, so P_r = w3^T(min(h,h_r) + M) needs ONE matmul.  The
    fused DVE scalar_tensor_tensor computes (h min h_r) add M in one ~660ns
    pass; this trades 244ns of PE for ~384ns of extra DVE per group and
    balances the engines (PE was the wall).  Only ONE row per group folds:
    the DVE is near-saturated (k=2 measured slower).  M is ~10x larger than
    h, so the whole kernel runs fp16 (10 mantissa bits; same PE/DVE rate as
    bf16) -- measured rel err ~1e-3.  Measured dead ends kept out: GPSIMD
    elementwise (SBUF-port contention inflates concurrent DVE ops ~60%),
    multi-bank PSUM "duo" tiles + batched 1024-col relus (ACT init does not
    amortize: 1197ns vs 2x597; the 3-duo ring stalls the PE), and the
    tensor_scalar (min,add) c-bias fold (+114ns/op on DVE for nothing once
    batching fails).
  - Encoder runs on-device in fp16 (all 512 nodes, replicated per core),
    fp32 PSUM accumulation, fp32 biases.  A2 = wp1a^T h + bp1 and M are
    precomputed once in the ramp.
  - The 64 rows are processed in 16 groups of 4, one row from each of the four
    16-row output blocks (i, i+16, i+32, i+48).  Per group (d tiles produced
    one group ahead; r2's stt first -- it is the DVE's slowest op):
      PE: w2p starts for r0,r1,r3 | w3 r2 (single) | outs g-2 | w3 stops
          r0,r1,r3  (acc pairs 5 slots apart so the same-bank accumulate
          never stalls on the PSUM drain)
      relus: r0,r1,r3 on ACT (bias=A2 col); r2 split ACT[:S]+DVE[S:] to
          balance the engines (ACT 3 full + piece ~= DVE mins+stt+piece).
  - out[16b+i,:] += embW_r^T hid_r: 4 col-tiled PE matmuls to PSUM partitions
    32b..32b+8 of the phase bank -- disjoint col_grp strips run concurrently
    (~1 slot for all 4).  embW_r = embbuf[:, 15-i : 31-i]: a sliding window
    over a 31-column zero buffer with Wp2 at column 15 puts Wp2 exactly in
    stationary column i.  Out matmuls of group g are issued in group g+2 so
    the in-order PE never waits on a relu.  Two 8-group phases accumulate
    into separate banks outp0/outp1 (PH=16 in one bank measured +219ns/group
    of PE on the out matmuls; separate banks also decouple the phase-0 flush
    from phase 1's first accumulate).
  - Output: bp2 is added on the host after the gather.  Phase-0 flush: copy
    halves on DVE in groups 12-13, DMAs on sync/gpsimd mid-steady.  Phase-1
    flush at the tail: copy halves on DVE+ACT in parallel, DMAs on
    sync/scalar (gpsimd is kept strictly off the tail: its SWDGE drain was
    measured at +9us there).
  - Startup: xtp is DMA'd in 5 k-chunks with doorbells spread across the sync/
    gpsimd queues (doorbells cost ~600ns each and serialize per queue);
    encoder matmuls start as chunks land, with narrow [0:128] first-pieces of
    relu1/h2/hbf so the hT/min chain launches early; small dummy matmuls
    bridge every DMA/relu gap so the PE HAM clock-gate stays at 2.4 GHz.
"""

import numpy as np

B = 512
IN_DIM = 595
HID = 128
NCORES = 8
RPC = B // NCORES  # rows per core = 64
NBLK = 4           # output col-tile blocks
BLK = RPC // NBLK  # 16 rows per block
DEFER_G = 2        # groups between producing hid and its out matmul
N_WARM_MM = 8      # dummy matmuls: sustained PE activity trips the HAM
                   # clock-gate to 2.4 GHz before the encoder matmuls run
WARM_N = 256       # free dim of warm matmuls
SPLIT_S = 112      # r2 relu split column: ACT [0:S], DVE [S:512]

# in_dim k-tiles for the first encoder matmul (contraction over 595)
KT = [(0, 128), (128, 256), (256, 384), (384, 512), (512, 595)]

_PROGRAM_CACHE = {}


def _build_program():
    import concourse.mybir as mybir
    import concourse.tile as tile
    from concourse import bacc

    f32 = mybir.dt.float32
    f16 = mybir.dt.float16
    Act = mybir.ActivationFunctionType
    Alu = mybir.AluOpType

    nc = bacc.Bacc("TRN2", target_bir_lowering=False)

    NKT = len(KT)
    xt_d = nc.dram_tensor("xtp", [HID, NKT * B], f16, kind="ExternalInput")
    w1_d = nc.dram_tensor("w1p", [HID, NKT * HID], f16, kind="ExternalInput")
    wpack_d = nc.dram_tensor("wpack", [HID, 5 * HID], f16, kind="ExternalInput")
    bias_d = nc.dram_tensor("biases", [HID, 5], f32, kind="ExternalInput")
    out_d = nc.dram_tensor("out", [RPC, B], f32, kind="ExternalOutput")

    with tile.TileContext(nc) as tc:
        with (
            tc.tile_pool(name="consts", bufs=1) as consts,
            tc.tile_pool(name="setup", bufs=1) as setup,
            tc.tile_pool(name="dwork", bufs=9) as dwork,
            tc.tile_pool(name="hwork", bufs=13) as hwork,
            tc.tile_pool(name="penc", bufs=1, space="PSUM") as penc,
            tc.tile_pool(name="ppair", bufs=7, space="PSUM") as ppair,
        ):
            # ---- input loads first: doorbells cost ~600ns each and serialize
            # per queue, so spread the xtp chunks across three idle queues.
            xt_all = consts.tile([HID, NKT * B], f16)
            w1_all = consts.tile([HID, NKT * HID], f16)
            biases = consts.tile([HID, 5], f32)
            wpack = consts.tile([HID, 5 * HID], f16)
            # earliest-needed first; k-chunks split across sync/gpsimd queues
            nc.scalar.dma_start(out=w1_all, in_=w1_d[:, :])
            qeng = [nc.sync, nc.gpsimd, nc.sync, nc.gpsimd, nc.sync]
            for k in range(NKT):
                qeng[k].dma_start(
                    out=xt_all[:, k * B : (k + 1) * B],
                    in_=xt_d[:, k * B : (k + 1) * B],
                )
            nc.scalar.dma_start(out=biases, in_=bias_d[:, :])
            nc.scalar.dma_start(out=wpack, in_=wpack_d[:, :])

            # ---- PE warm-up over the DMA window (HAM ramps to 2.4 GHz)
            scratch = setup.tile([HID, B], f16)
            nc.vector.memset(scratch, 0.0)
            scratch1 = setup.tile([HID, 1], f32)
            nc.scalar.activation(scratch1, scratch[:, 0:1], Act.Relu)

            def warm_mm(n, w=WARM_N):
                # dummy matmuls keep the PE busy (HAM clock-gate stays at
                # 2.4 GHz) across DMA-wait and relu-wait gaps; they use ppair
                # banks so they never touch the encoder/out accumulator banks
                for _ in range(n):
                    wp = ppair.tile([HID, B], f32, name="pp")
                    nc.tensor.matmul(
                        wp[:, 0:w], lhsT=scratch[:, 0:HID], rhs=scratch[:, 0:w],
                        start=True, stop=True, skip_group_check=True,
                    )

            warm_mm(N_WARM_MM)

            # sliding-window Wp2 buffer: zeros with Wp2 at column BLK-1; the
            # out matmul for block-row i uses embbuf[:, BLK-1-i+c] == Wp2 iff
            # c == i.
            embbuf = consts.tile([HID, 2 * BLK - 1], f16)
            nc.vector.memset(embbuf, 0.0)

            xt_sb = [xt_all[:, k * B : (k + 1) * B] for k in range(NKT)]
            w1_sb = [w1_all[:, k * HID : (k + 1) * HID] for k in range(NKT)]
            w2_sb = wpack[:, 0 * HID : 1 * HID]
            wp1a_sb = wpack[:, 1 * HID : 2 * HID]
            w2p_sb = wpack[:, 2 * HID : 3 * HID]
            w3_sb = wpack[:, 3 * HID : 4 * HID]
            g1_sb = wpack[:, 4 * HID : 5 * HID]
            b1_sb = biases[:, 0:1]
            b2_sb = biases[:, 1:2]
            bp1_sb = biases[:, 2:3]

            # ---- encoder: h1 = relu(W1^T xT + b1), hT = relu(W2^T h1 + b2) ----
            h1p = penc.tile([HID, B], f32, name="encp", tag="encp")
            for k in range(len(KT)):
                nc.tensor.matmul(
                    h1p, lhsT=w1_sb[k], rhs=xt_sb[k],
                    start=(k == 0), stop=(k == len(KT) - 1),
                )
                if k > 0:
                    warm_mm(1)  # bridge the DMA-gated gaps between k-chunks
            # encoder relus split: a narrow ACT first-piece [0:128] lets the
            # h2 -> hbf -> hT -> min chain start early; DVE takes the rest
            HQ = B // 4
            HB2 = B // 2
            h1bf = setup.tile([HID, B], f16)
            nc.scalar.activation(h1bf[:, 0:HQ], h1p[:, 0:HQ], Act.Relu, bias=b1_sb)
            nc.vector.tensor_scalar(
                h1bf[:, HQ:B], h1p[:, HQ:B], b1_sb, 0.0, Alu.add, Alu.max
            )

            # h2 in two matmuls to different PSUM banks: a narrow first piece
            # [0:128] feeding ACT, the rest feeding DVE, so the hT/min chain
            # starts as early as possible.  h2p lives in a ppair bank: the
            # penc bank is still being read by h1bf's DVE piece, and a ppair
            # slot lets h2p start ~0.4us earlier (right after the ACT piece).
            h2p = ppair.tile([HID, HQ], f32, name="pp")
            nc.tensor.matmul(
                h2p, lhsT=w2_sb, rhs=h1bf[:, 0:HQ],
                start=True, stop=True, skip_group_check=True,
            )
            h2pb = ppair.tile([HID, B], f32, name="pp")
            nc.tensor.matmul(
                h2pb[:, 0 : B - HQ], lhsT=w2_sb, rhs=h1bf[:, HQ:B],
                start=True, stop=True, skip_group_check=True,
            )
            warm_mm(2)  # bridge PE over relu2 + hT
            hbf = setup.tile([HID, B], f16)
            nc.scalar.activation(hbf[:, 0:HQ], h2p, Act.Relu, bias=b2_sb)
            nc.vector.tensor_scalar(
                hbf[:, HQ:B], h2pb[:, 0 : B - HQ], b2_sb, 0.0, Alu.add, Alu.max
            )
            # hT fp32 is the per-row scalar operand of the min (tensor_scalar
            # scalars must be fp32); only the core's 64 local-row columns are
            # ever read, and deriving it from hbf avoids a second serialized
            # read of the h2p PSUM bank
            hT = setup.tile([HID, RPC], f32)
            nc.vector.tensor_copy(hT, hbf[:, 0:RPC])
            # Wp2 rides in as f32 column 4 of biases; cast into the sliding
            # window buffer.  Emitted HERE (needed only by the first out
            # matmuls): emitting it earlier head-of-line-blocks the in-order
            # DVE queue on the biases DMA and delays the encoder relus ~1us.
            nc.vector.tensor_copy(embbuf[:, BLK - 1 : BLK], biases[:, 4:5])

            # ---- M = (w2p w3^{-1})^T h FIRST (its fp16 copy gates the fold
            # rows' stt), then A2 = wp1a^T h + bp1.  mp is a transient ppair
            # tile so the penc bank stays free for outp.
            mp = ppair.tile([HID, B], f32, name="pp")
            nc.tensor.matmul(
                mp, lhsT=g1_sb, rhs=hbf,
                start=True, stop=True, skip_group_check=True,
            )
            a2p = penc.tile([HID, B], f32, name="encp3", tag="encp")
            nc.tensor.matmul(a2p, lhsT=wp1a_sb, rhs=hbf, start=True, stop=True)
            warm_mm(1)  # bridge PE over the first mins
            msb = setup.tile([HID, B], f16)
            nc.vector.tensor_copy(msb, mp)
            a2 = setup.tile([HID, B], f32)
            nc.scalar.activation(a2, a2p, Act.Identity, bias=bp1_sb)

            # ---- per-row d production, one group of lookahead ----
            # r2's fused stt first: it is the DVE's slowest op and feeds the
            # group's 5th matmul; plain mins feed the 6th-8th.  The first
            # FOLD_START groups run fully PE-style (plain mins only): their
            # d tiles are needed before the M -> msb chain completes.
            FOLD_START = 2
            dtiles = {}

            def emit_d(g):
                if not (0 <= g < BLK):
                    return
                r0, r1, r2, r3 = rows_of(g)
                if g >= FOLD_START:
                    dp = dwork.tile([HID, B], f16, name="dtile")
                    nc.vector.scalar_tensor_tensor(
                        dp, hbf, hT[:, r2 : r2 + 1], msb, Alu.min, Alu.add
                    )
                    dtiles[r2] = dp
                    plain = (r0, r1, r3)
                else:
                    plain = (r0, r1, r2, r3)
                for r in plain:
                    d = dwork.tile([HID, B], f16, name="dtile")
                    nc.vector.tensor_scalar(
                        d, hbf, hT[:, r : r + 1], None, Alu.min
                    )
                    dtiles[r] = d

            # out accumulation in two phases of 8 groups each, reusing ONE
            # PSUM bank (the encoder bank): phase p group g writes partition
            # 32b + (g - 8p) of block b's col_grp strip; the phase-0 flush
            # (copy to SBUF, 4 strip DMAs) overlaps the phase-1 compute.  The
            # 4 blocks hit disjoint col_grp strips of the PE array and their
            # out matmuls run concurrently.
            PH = BLK // 2  # 8 groups per phase
            outp = penc.tile([HID, B], f32, name="outp", tag="encp")

            def rows_of(g):
                return [g + BLK * b for b in range(NBLK)] if 0 <= g < BLK else []

            pending = {}

            def emit_outs(g):
                go = g % PH
                for b in range(NBLK):
                    r = g + BLK * b
                    hid_r = pending.pop(r)
                    nc.tensor.matmul(
                        outp[32 * b : 32 * b + PH, :],
                        lhsT=embbuf[:, BLK - 1 - go : BLK - 1 - go + PH],
                        rhs=hid_r,
                        start=(go == 0), stop=(go == PH - 1),
                        skip_group_check=True,
                        tile_position=(0, 32 * b),
                    )

            flush_state = {}

            def emit_flush_copy(p, half, eng):
                # copy PSUM -> SBUF in two column-halves (bp2 is added on the
                # host after the gather); ACT has no tensor_copy, so it uses
                # an Identity activation
                if p not in flush_state:
                    flush_state[p] = setup.tile([HID, B], f32, name=f"outs{p}")
                o = flush_state[p]
                sl = slice(half * HB2, (half + 1) * HB2)
                if eng is nc.scalar:
                    eng.activation(o[:, sl], outp[:, sl], Act.Identity)
                else:
                    eng.tensor_copy(o[:, sl], outp[:, sl])

            def emit_flush_dmas(p):
                # phase-0 descgen on sync/gpsimd (mid-steady; the gpsimd
                # sequencer is idle); phase-1 at the tail on sync/scalar
                # (gpsimd is kept strictly off the tail: measured +9us
                # regression from its SWDGE drain there)
                o = flush_state[p]
                fq = (
                    [nc.sync, nc.gpsimd, nc.sync, nc.gpsimd]
                    if p == 0
                    else [nc.sync, nc.scalar, nc.sync, nc.scalar]
                )
                for b in range(NBLK):
                    fq[b].dma_start(
                        out=out_d[BLK * b + PH * p : BLK * b + PH * (p + 1), :],
                        in_=o[32 * b : 32 * b + PH, :],
                    )

            # prime the d pipeline
            emit_d(0)

            # ---- pairwise main loop: 16 groups of 4 rows ----
            for g in range(BLK):
                r0, r1, r2, r3 = rows_of(g)
                emit_d(g + 1)
                # The second flush-copy half must precede phase-1's first out
                # matmuls (start=True clears the shared bank).
                if g - DEFER_G == PH:
                    emit_flush_copy(0, 1, nc.vector)
                    emit_flush_dmas(0)
                fold = g >= FOLD_START
                acc_rows = (r0, r1, r3) if fold else (r0, r1, r2, r3)
                pps = {}
                for r in acc_rows:
                    pp = ppair.tile([HID, B], f32, name="pp")
                    nc.tensor.matmul(
                        pp, lhsT=w2p_sb, rhs=hbf,
                        start=True, stop=False, skip_group_check=True,
                    )
                    pps[r] = pp
                # deferred out matmuls (4 col-tiled, concurrent) sit between
                # the w2p starts and the w3 block: their post-drain penalty
                # overlaps r2's stt wait, the w3 stationary is loaded once for
                # all 4-5 w3 matmuls, and the same-bank acc pairs get 5 slots
                # of spacing
                if g - DEFER_G >= 0:
                    emit_outs(g - DEFER_G)
                    if g - DEFER_G == PH - 1:
                        emit_flush_copy(0, 0, nc.vector)
                if fold:
                    pp2 = ppair.tile([HID, B], f32, name="pp")
                    nc.tensor.matmul(
                        pp2, lhsT=w3_sb, rhs=dtiles.pop(r2),
                        start=True, stop=True, skip_group_check=True,
                    )
                    pps[r2] = pp2
                for r in acc_rows:
                    nc.tensor.matmul(
                        pps[r], lhsT=w3_sb, rhs=dtiles.pop(r),
                        start=False, stop=True, skip_group_check=True,
                    )
                # relus: r2 split ACT[0:S] + DVE[S:], r0/r1/r3 full on ACT.
                # r2's pieces can issue early (its matmul is the group's 4th
                # slot); in the last group there is no next-group d work, so
                # give the DVE a full relu to shorten the ACT chain.
                hid2 = hwork.tile([HID, B], f16, name="hid")
                pp2r = pps[r2]
                if g == BLK - 1:
                    nc.vector.tensor_scalar(
                        hid2, pp2r, a2[:, r2 : r2 + 1], 0.0, Alu.add, Alu.max
                    )
                else:
                    nc.scalar.activation(
                        hid2[:, 0:SPLIT_S], pp2r[:, 0:SPLIT_S], Act.Relu,
                        bias=a2[:, r2 : r2 + 1],
                    )
                    nc.vector.tensor_scalar(
                        hid2[:, SPLIT_S:B], pp2r[:, SPLIT_S:B],
                        a2[:, r2 : r2 + 1], 0.0, Alu.add, Alu.max,
                    )
                pending[r2] = hid2
                for r in (r0, r1, r3):
                    hid = hwork.tile([HID, B], f16, name="hid")
                    nc.scalar.activation(
                        hid, pps[r], Act.Relu, bias=a2[:, r : r + 1]
                    )
                    pending[r] = hid
            for g in range(BLK - DEFER_G, BLK):
                emit_outs(g)
            # tail flush: ONE full copy on DVE (it sees the PE's stop ~500ns
            # before ACT does), leaving the scalar queue free to start its
            # DMA descriptor generation the moment the copy lands
            flush_state[1] = setup.tile([HID, B], f32, name="outs1")
            nc.vector.tensor_copy(flush_state[1], outp)
            emit_flush_dmas(1)

    nc.finalize()
    return nc


def _get_program():
    if "nc" not in _PROGRAM_CACHE:
        _PROGRAM_CACHE["nc"] = _build_program()
    return _PROGRAM_CACHE["nc"]


def _make_in_maps(x, W1, b1, W2, b2, Wp1, bp1, Wp2, bp2):
    f16 = np.float16
    f32 = np.float32
    f64 = np.float64
    x = np.asarray(x, dtype=f32)
    W1 = np.asarray(W1, dtype=f32)
    W2 = np.asarray(W2, dtype=f32)
    Wp1 = np.asarray(Wp1, dtype=f64)
    Wp2 = np.asarray(Wp2, dtype=f32).reshape(HID, 1)
    b1c = np.ascontiguousarray(np.asarray(b1, dtype=f32).reshape(HID, 1))
    b2c = np.ascontiguousarray(np.asarray(b2, dtype=f32).reshape(HID, 1))
    bp1c = np.ascontiguousarray(np.asarray(bp1, dtype=f32).reshape(HID, 1))

    # |h_i - h_j| = h_i + h_j - 2*min(h_i, h_j) folds (see module docstring)
    w3f = Wp1[2 * HID : 3 * HID, :]
    wp1a = Wp1[0:HID, :] + w3f
    w2p = Wp1[HID : 2 * HID, :] + w3f
    w3 = -2.0 * w3f
    # w3^{-1} fold: M = G1^T h adds the w2p^T h term through the w3 matmul
    G1 = w2p @ np.linalg.inv(w3)

    NKT = len(KT)
    KPAD = NKT * HID  # 640: in_dim padded so every k-tile is 128 partitions

    # packed weights [w2 | wp1a | w2p | w3 | G1],
    # biases [b1 | b2 | bp1 | bp2 | wp2] (bp2 unused on device)
    wpack = np.concatenate(
        [W2.astype(f64), wp1a, w2p, w3, G1], axis=1
    ).astype(f16)
    biases = np.zeros((HID, 5), dtype=f32)
    biases[:, 0:1] = b1c
    biases[:, 1:2] = b2c
    biases[:, 2:3] = bp1c
    biases[:, 4:5] = Wp2

    # w1 padded to [640, 128], viewed as [128, 5*128]
    w1_pad = np.zeros((KPAD, HID), dtype=f32)
    w1_pad[:IN_DIM] = np.asarray(W1, dtype=f32)
    w1p = np.ascontiguousarray(
        w1_pad.reshape(NKT, HID, HID).transpose(1, 0, 2).reshape(HID, NKT * HID)
    ).astype(f16)

    shared = dict(w1p=w1p, wpack=wpack, biases=biases)
    in_maps = []
    for c in range(NCORES):
        xr = np.roll(x, -c * RPC, axis=0)
        xt_pad = np.zeros((KPAD, B), dtype=f32)
        xt_pad[:IN_DIM] = xr.T
        xtp = np.ascontiguousarray(
            xt_pad.reshape(NKT, HID, B).transpose(1, 0, 2).reshape(HID, NKT * B)
        ).astype(f16)
        m = dict(shared)
        m["xtp"] = xtp
        in_maps.append(m)
    return in_maps


def _run(in_maps, trace=False):
    from concourse.bass_utils import run_bass_kernel_spmd

    nc = _get_program()
    return run_bass_kernel_spmd(
        nc, in_maps, core_ids=list(range(NCORES)), trace=trace
    )


def kernel(x, W1, b1, W2, b2, Wp1, bp1, Wp2, bp2):
    in_maps = _make_in_maps(x, W1, b1, W2, b2, Wp1, bp1, Wp2, bp2)
    res = _run(in_maps, trace=False)
    bp2v = np.float32(np.asarray(bp2, dtype=np.float32).reshape(-1)[0])
    out = np.empty((B, B), dtype=np.float32)
    for c in range(NCORES):
        blk = np.asarray(res.results[c]["out"], dtype=np.float32)
        # device block row r*BLK.. maps rows (g + BLK*b); device row order is
        # [g + 16b] = natural order, so rows are already 0..63
        out[c * RPC : (c + 1) * RPC, :] = np.roll(blk, c * RPC, axis=1) + bp2v
    return out
